# revision 15
# baseline (speedup 1.0000x reference)
"""Trainium2 Bass kernel for nn_ContinuousOutputGenerator.

Math (per batch element b):
    proj = gelu(states @ W1 + b1) @ W2 + b2                      [N, O]
    w_u[n, g=(i,j)] = exp(-((gx_i-px_n)^2 + (gy_j-py_n)^2)/bw)   [N, G]
    out[g, :] = sum_n w_u[n, g] * proj[n, :] / (sum_n w_u[n, g] + eps)

Key algebraic restructuring:
  * The RBF kernel matrix is SEPARABLE over the 64x64 grid:
        w_u[n, (i,j)] = A[n,i] * B[n,j]
    A/B are computed in ONE activation each via Derivative_Erf
    (d/dx erf = 2/sqrt(pi) * exp(-x^2)); the constant squares into both
    numerator and normalizer, so it cancels after normalization provided
    eps is scaled by 4/pi.
  * The normalizer S[(i,j)] = (A^T @ B)[i,j] is a single small accumulated
    matmul; normalization is deferred to a per-row scale of the pooled
    output on the Scalar engine.

Performance structure (vs. naive):
  * All matmul operands in bf16: halves LDWEIGHTS time and DVE outer-
    product time. PSUM accumulation stays fp32.
  * Activation functions are phase-separated so the ACT table is loaded
    twice total (derivative_erf set, then gelu set) instead of
    thrashing per chunk.
  * states are shipped pre-transposed [D, N] so no PE transposes needed.
  * Pooling output scale-by-1/(S+eps) runs on the Scalar engine (Copy
    with scale AP), keeping DVE free for the wu outer products.

Sharding: data-parallel over batch. 8 batch elements -> 8 NeuronCores, MLP
weights replicated. Each core runs the identical program on its own slice.
"""

import os
import sys
from contextlib import ExitStack

import numpy as np

if "/opt/trn_rl_repo" not in sys.path:
    sys.path.insert(0, "/opt/trn_rl_repo")

import concourse.bass as bass  # noqa: E402
import concourse.tile as tile  # noqa: E402
from concourse import bacc, bass_utils, mybir  # noqa: E402

F32 = mybir.dt.float32
F32R = mybir.dt.float32r
BF16 = mybir.dt.bfloat16
AF = mybir.ActivationFunctionType

# Problem shape (hardcoded per contract)
B, N, D, H, O = 8, 4096, 256, 512, 256
GRID = 64
G = GRID * GRID
NT = N // 128          # 32 n-tiles of 128 entities
NCHUNK = 8             # MLP processes n in chunks of 512
CSUB = 4               # 128-row subtiles per chunk
GCHUNK = 4             # pooling g-chunks of 1024 grid points
GG = G // GCHUNK       # 1024
IPC = GRID // GCHUNK   # 16 i-values per g-chunk
BW = 0.1
EPS = 1e-8

# A/B via single Derivative_Erf (True) or Square+Exp pair (False).
# (env override is a dev knob; CoreSim lacks Derivative_Erf)
USE_DERF = os.environ.get("KERNEL_USE_DERF", "1") == "1"
# 2/sqrt(pi) from d/dx erf; A and B each carry it, so S carries its square.
DERF_C2 = 4.0 / np.pi


def _body(tc, aps, out_ap):
    nc = tc.nc
    with ExitStack() as ctx:
        # ---------------- persistent SBUF ----------------
        const = ctx.enter_context(tc.tile_pool(name="const", bufs=1))
        w1k = [const.tile([128, H], BF16, tag=f"w1k{k}", name=f"w1k{k}") for k in range(2)]
        w2k = [const.tile([128, O], BF16, tag=f"w2k{k}", name=f"w2k{k}") for k in range(4)]
        stb = [const.tile([128, N], BF16, tag=f"stb{k}", name=f"stb{k}") for k in range(2)]
        gridb_sb = const.tile([128, GRID], F32, tag="gridb")
        negpos_sb = const.tile([128, 2 * NT], F32, tag="negpos")
        b2b_sb = const.tile([128, O], F32, tag="b2b")
        b1_sb = const.tile([128, 4], F32, tag="b1")
        s_sb = const.tile([GRID, GRID], F32, tag="s_sb")
        r_sb = const.tile([GRID, GRID], F32, tag="r_sb")
        r_t = const.tile([128, NT], F32, tag="r_t")

        ab = ctx.enter_context(tc.tile_pool(name="ab", bufs=1))
        a_all = ab.tile([128, NT * GRID], BF16, tag="a_all")
        b_all = ab.tile([128, NT * GRID], BF16, tag="b_all")

        projp = ctx.enter_context(tc.tile_pool(name="projp", bufs=1))
        proj = projp.tile([128, NT * O], BF16, tag="proj")

        dram = ctx.enter_context(tc.tile_pool(name="dram", bufs=1, space="DRAM"))
        scr = dram.tile([G], F32, tag="scr")

        # ---------------- const + states DMAs (issue first, overlap) ----
        nc.sync.dma_start(gridb_sb[:], aps["gridb"][:])
        nc.sync.dma_start(negpos_sb[:], aps["negpos"][:])
        # states and MLP weights arrive pre-cast to bf16 from the host: no
        # on-device cast pass, half the DMA bytes, MM1 starts on DMA landing.
        for k in range(2):
            nc.sync.dma_start(stb[k][:], aps["statesT"][k * 128 : (k + 1) * 128, :])
        for k in range(2):
            nc.sync.dma_start(w1k[k][:], aps["W1"][k * 128 : (k + 1) * 128, :])
        for k in range(4):
            nc.sync.dma_start(w2k[k][:], aps["W2"][k * 128 : (k + 1) * 128, :])
        nc.sync.dma_start(b2b_sb[:], aps["b2b"][:])
        nc.sync.dma_start(b1_sb[:], aps["b1"].rearrange("(m p) -> p m", p=128))

        # ---------------- phase A: RBF factors + normalizer ----------------
        # A[n,i] = c*exp(-(gx_i - px_n)^2 / bw), B likewise for y (c=2/sqrt(pi)
        # when USE_DERF; the c^2 in numerator and S cancels, eps is scaled).
        tmp = ctx.enter_context(tc.tile_pool(name="tmp", bufs=4))
        for a in range(NT):
            for h_or_v in range(2):  # 0 -> A (x), 1 -> B (y)
                dst = (a_all if h_or_v == 0 else b_all)[:, a * GRID : (a + 1) * GRID]
                bias_ap = negpos_sb[:, 2 * a + h_or_v : 2 * a + h_or_v + 1]
                if USE_DERF:
                    nc.scalar.activation(
                        dst, gridb_sb[:], AF.Derivative_Erf,
                        bias=bias_ap, scale=1.0 / np.sqrt(BW),
                    )
                else:
                    t = tmp.tile([128, GRID], F32, tag="sq")
                    nc.scalar.activation(t[:], gridb_sb[:], AF.Square, bias=bias_ap)
                    nc.scalar.activation(dst, t[:], AF.Exp, scale=-1.0 / BW)

        # normalizer S = A^T @ B (fp32 accum), R = 1/(S+eps)
        with tc.tile_pool(name="ps_s", bufs=1, space="PSUM") as ps_s:
            ps = ps_s.tile([GRID, GRID], F32, tag="ps_s")
            for a in range(NT):
                nc.tensor.matmul(
                    ps[:],
                    a_all[:, a * GRID : (a + 1) * GRID],
                    b_all[:, a * GRID : (a + 1) * GRID],
                    start=(a == 0),
                    stop=(a == NT - 1),
                )
            eps_eff = EPS * DERF_C2 if USE_DERF else EPS
            nc.vector.tensor_scalar_add(s_sb[:], ps[:], eps_eff)
        nc.vector.reciprocal(r_sb[:], s_sb[:])
        # repartition R [64i, 64j] -> [128 part, 32 g-tiles] via DRAM
        nc.sync.dma_start(scr[:].rearrange("(i j) -> i j", i=GRID), r_sb[:])
        nc.sync.dma_start(r_t[:], scr[:].rearrange("(t p) -> p t", p=128))

        # ---------------- phase B: MLP ----------------
        hT = ctx.enter_context(tc.tile_pool(name="hT", bufs=2))
        with (
            tc.tile_pool(name="ps_h", bufs=2, space="PSUM") as ps_h,
            tc.tile_pool(name="ps_p", bufs=2, space="PSUM") as ps_p,
        ):
            for c in range(NCHUNK):
                n0 = c * 512
                # MM1 + exact GELU: hT[m] = gelu(W1^T statesT + b1), [h=512,n=512]
                hts = [hT.tile([128, 512], BF16, tag=f"hT{m}", name=f"hT{m}") for m in range(4)]
                for m in range(4):
                    ph = ps_h.tile([128, 512], F32, tag="ph")
                    for k in range(2):
                        nc.tensor.matmul(
                            ph[:],
                            w1k[k][:, m * 128 : (m + 1) * 128],
                            stb[k][:, n0 : n0 + 512],
                            start=(k == 0),
                            stop=(k == 1),
                        )
                    nc.scalar.activation(
                        hts[m][:], ph[:], AF.Gelu, bias=b1_sb[:, m : m + 1]
                    )
                # MM2 + bias: proj[n_tile] = hT^T W2 + b2, [n=128, o=256]
                for s in range(CSUB):
                    a = c * CSUB + s
                    pp = ps_p.tile([128, O], F32, tag="pp")
                    for k in range(4):
                        nc.tensor.matmul(
                            pp[:],
                            hts[k][:, s * 128 : (s + 1) * 128],
                            w2k[k][:],
                            start=(k == 0),
                            stop=(k == 3),
                        )
                    nc.vector.tensor_add(
                        proj[:, a * O : (a + 1) * O], pp[:], b2b_sb[:]
                    )

        # ---------------- phase C: pooling out = (w_u^T proj) * R ----------
        wup = ctx.enter_context(tc.tile_pool(name="wup", bufs=8))
        osbp = ctx.enter_context(tc.tile_pool(name="osbp", bufs=4))
        with tc.tile_pool(name="ps_acc", bufs=1, space="PSUM") as ps_acc:
            for gc in range(GCHUNK):
                accs = [ps_acc.tile([128, 512], F32, tag=f"acc{t}", name=f"acc{t}") for t in range(4)]
                for a in range(NT):
                    wu = wup.tile([128, GG], BF16, tag="wu")
                    i0 = a * GRID + gc * IPC
                    a3 = a_all[:, i0 : i0 + IPC][:, :, None].broadcast_to(
                        [128, IPC, GRID]
                    )
                    b3 = b_all[:, a * GRID : (a + 1) * GRID][:, None, :].broadcast_to(
                        [128, IPC, GRID]
                    )
                    wu3 = wu[:].rearrange("p (i j) -> p i j", i=IPC)
                    # split outer-product builds across DVE and GpSimd —
                    # both run ~1 elem/lane/ns, and PE is the phase bottleneck
                    eng = nc.vector if a % 2 == 0 else nc.gpsimd
                    eng.tensor_mul(wu3, a3, b3)
                    for m in range(8):
                        # start=True clears the whole PSUM bank, so only the
                        # first matmul into each bank may set it; the second
                        # half lands on cleared has_written bits and overwrites.
                        nc.tensor.matmul(
                            accs[m // 2][:, (m % 2) * O : (m % 2 + 1) * O],
                            wu[:, m * 128 : (m + 1) * 128],
                            proj[:, a * O : (a + 1) * O],
                            start=(a == 0 and m % 2 == 0),
                            stop=(a == NT - 1),
                        )
                for t in range(4):
                    osb = osbp.tile([128, 512], F32, tag="osb")
                    for half in range(2):
                        gt = gc * 8 + t * 2 + half
                        nc.scalar.activation(
                            osb[:, half * O : (half + 1) * O],
                            accs[t][:, half * O : (half + 1) * O],
                            AF.Copy,
                            scale=r_t[:, gt : gt + 1],
                        )
                    r0 = (gc * 4 + t) * 256
                    nc.sync.dma_start(
                        out_ap[r0 : r0 + 256, :].rearrange("(a p) o -> p a o", a=2),
                        osb[:].rearrange("p (a o) -> p a o", a=2),
                    )


def build_module():
    nc = bacc.Bacc("TRN2", target_bir_lowering=False, debug=False, num_devices=B)
    aps = {
        "statesT": nc.dram_tensor("statesT", (D, N), BF16, kind="ExternalInput").ap(),
        "W1": nc.dram_tensor("W1", (D, H), BF16, kind="ExternalInput").ap(),
        "b1": nc.dram_tensor("b1", (H,), F32, kind="ExternalInput").ap(),
        "W2": nc.dram_tensor("W2", (H, O), BF16, kind="ExternalInput").ap(),
        "b2b": nc.dram_tensor("b2b", (128, O), F32, kind="ExternalInput").ap(),
        "gridb": nc.dram_tensor("gridb", (128, GRID), F32, kind="ExternalInput").ap(),
        "negpos": nc.dram_tensor(
            "negpos", (128, 2 * NT), F32, kind="ExternalInput"
        ).ap(),
    }
    out_ap = nc.dram_tensor("out", (G, O), F32, kind="ExternalOutput").ap()
    with tile.TileContext(nc) as tc:
        _body(tc, aps, out_ap)
    nc.compile()
    return nc


_NC = None


def _get_nc():
    global _NC
    if _NC is None:
        _NC = build_module()
    return _NC


def make_in_maps(inputs):
    bf16 = mybir.dt.np(BF16)
    states = np.asarray(inputs["entity_states"], np.float32)
    pos = np.asarray(inputs["entity_positions"], np.float32)
    W1 = np.ascontiguousarray(np.asarray(inputs["W1"], np.float32).astype(bf16))
    b1 = np.ascontiguousarray(np.asarray(inputs["b1"], np.float32))
    W2 = np.ascontiguousarray(np.asarray(inputs["W2"], np.float32).astype(bf16))
    b2 = np.asarray(inputs["b2"], np.float32)

    statesT = np.ascontiguousarray(states.transpose(0, 2, 1).astype(bf16))  # [B,D,N]
    g = np.linspace(-1.0, 1.0, GRID).astype(np.float32)
    gridb = np.ascontiguousarray(np.tile(g[None, :], (128, 1)))
    b2b = np.ascontiguousarray(np.tile(b2[None, :], (128, 1)))
    # negpos[p, 2a+c] = -pos[a*128+p, c] (scaled for the d_erf argument)
    npz = (-pos).reshape(B, NT, 128, 2).transpose(0, 2, 1, 3).reshape(B, 128, 2 * NT)
    if USE_DERF:
        npz = npz / np.sqrt(BW)
    negpos = np.ascontiguousarray(npz.astype(np.float32))
    return [
        {
            "statesT": statesT[b],
            "W1": W1,
            "b1": b1,
            "W2": W2,
            "b2b": b2b,
            "gridb": gridb,
            "negpos": negpos[b],
        }
        for b in range(B)
    ]


def run(inputs, trace=False, **kw):
    nc = _get_nc()
    res = bass_utils.run_bass_kernel_spmd(
        nc, make_in_maps(inputs), core_ids=list(range(B)), trace=trace, **kw
    )
    out = np.stack([r["out"] for r in res.results], axis=0)
    return out, res


def kernel(**inputs) -> np.ndarray:
    out, _ = run(inputs, trace=False)
    return out


# revision 22
# speedup vs baseline: 1.1512x; 1.1512x over previous
"""Trainium2 Bass kernel for nn_ContinuousOutputGenerator.

Math (per batch element b):
    proj = gelu(states @ W1 + b1) @ W2 + b2                      [N, O]
    w_u[n, g=(i,j)] = exp(-((gx_i-px_n)^2 + (gy_j-py_n)^2)/bw)   [N, G]
    out[g, :] = sum_n w_u[n, g] * proj[n, :] / (sum_n w_u[n, g] + eps)

Key algebraic restructuring:
  * The RBF kernel matrix is SEPARABLE over the 64x64 grid:
        w_u[n, (i,j)] = A[n,i] * B[n,j]
    A/B are computed in ONE activation each via Derivative_Erf
    (d/dx erf = 2/sqrt(pi) * exp(-x^2)); the constant squares into both
    numerator and normalizer, so it cancels after normalization provided
    eps is scaled by 4/pi.
  * The normalizer S[(i,j)] = (A^T @ B)[i,j] is a single small accumulated
    matmul; normalization is deferred to a per-row scale of the pooled
    output on the Scalar engine.

Performance structure (vs. naive):
  * All matmul operands in bf16: halves LDWEIGHTS time and DVE outer-
    product time. PSUM accumulation stays fp32.
  * Activation functions are phase-separated so the ACT table is loaded
    twice total (derivative_erf set, then gelu set) instead of
    thrashing per chunk.
  * states are shipped pre-transposed [D, N] so no PE transposes needed.
  * Pooling output scale-by-1/(S+eps) runs on the Scalar engine (Copy
    with scale AP), keeping DVE free for the wu outer products.

Sharding: data-parallel over batch. 8 batch elements -> 8 NeuronCores, MLP
weights replicated. Each core runs the identical program on its own slice.
"""

import os
import sys
from contextlib import ExitStack

import numpy as np

if "/opt/trn_rl_repo" not in sys.path:
    sys.path.insert(0, "/opt/trn_rl_repo")

import concourse.bass as bass  # noqa: E402
import concourse.tile as tile  # noqa: E402
from concourse import bacc, bass_utils, mybir  # noqa: E402

F32 = mybir.dt.float32
F32R = mybir.dt.float32r
BF16 = mybir.dt.bfloat16
AF = mybir.ActivationFunctionType

# Problem shape (hardcoded per contract)
B, N, D, H, O = 8, 4096, 256, 512, 256
GRID = 64
G = GRID * GRID
NT = N // 128          # 32 n-tiles of 128 entities
NCHUNK = 8             # MLP processes n in chunks of 512
CSUB = 4               # 128-row subtiles per chunk
GCHUNK = 4             # pooling g-chunks of 1024 grid points
GG = G // GCHUNK       # 1024
IPC = GRID // GCHUNK   # 16 i-values per g-chunk
BW = 0.1
EPS = 1e-8

# A/B via single Derivative_Erf (True) or Square+Exp pair (False).
# (env override is a dev knob; CoreSim lacks Derivative_Erf)
USE_DERF = os.environ.get("KERNEL_USE_DERF", "1") == "1"
# 2/sqrt(pi) from d/dx erf; A and B each carry it, so S carries its square.
DERF_C2 = 4.0 / np.pi


def _body(tc, aps, out_ap):
    nc = tc.nc
    with ExitStack() as ctx:
        # ---------------- persistent SBUF ----------------
        const = ctx.enter_context(tc.tile_pool(name="const", bufs=1))
        w1k = [const.tile([128, H], BF16, tag=f"w1k{k}", name=f"w1k{k}") for k in range(2)]
        w2k = [const.tile([128, O], BF16, tag=f"w2k{k}", name=f"w2k{k}") for k in range(4)]
        stb = [const.tile([128, N], BF16, tag=f"stb{k}", name=f"stb{k}") for k in range(2)]
        identb = const.tile([128, 128], BF16, tag="identb")
        gridb_sb = const.tile([128, GRID], F32, tag="gridb")
        negpos_sb = const.tile([128, 2 * NT], F32, tag="negpos")
        b2b_sb = const.tile([128, O], F32, tag="b2b")
        b1_sb = const.tile([128, 4], F32, tag="b1")
        s_sb = const.tile([GRID, GRID], F32, tag="s_sb")
        r_sb = const.tile([GRID, GRID], F32, tag="r_sb")
        r_t = const.tile([128, NT], F32, tag="r_t")

        ab = ctx.enter_context(tc.tile_pool(name="ab", bufs=1))
        a_all = ab.tile([128, NT * GRID], BF16, tag="a_all")
        b_all = ab.tile([128, NT * GRID], BF16, tag="b_all")

        projp = ctx.enter_context(tc.tile_pool(name="projp", bufs=1))
        proj = projp.tile([128, NT * O], BF16, tag="proj")

        dram = ctx.enter_context(tc.tile_pool(name="dram", bufs=1, space="DRAM"))
        scr = dram.tile([G], F32, tag="scr")

        # ---------------- const + states DMAs (issue first, overlap) ----
        nc.sync.dma_start(gridb_sb[:], aps["gridb"][:])
        nc.sync.dma_start(negpos_sb[:], aps["negpos"][:])
        # states and MLP weights arrive pre-cast to bf16 from the host: no
        # on-device cast pass, half the DMA bytes, MM1 starts on DMA landing.
        for k in range(2):
            nc.sync.dma_start(stb[k][:], aps["statesT"][k * 128 : (k + 1) * 128, :])
        for k in range(2):
            nc.sync.dma_start(w1k[k][:], aps["W1"][k * 128 : (k + 1) * 128, :])
        for k in range(4):
            nc.sync.dma_start(w2k[k][:], aps["W2"][k * 128 : (k + 1) * 128, :])
        nc.sync.dma_start(b2b_sb[:], aps["b2b"][:])
        nc.sync.dma_start(b1_sb[:], aps["b1"].rearrange("(m p) -> p m", p=128))
        nc.sync.dma_start(identb[:], aps["ident"][:])

        # ---------------- phase A: RBF factors + normalizer ----------------
        # A[n,i] = c*exp(-(gx_i - px_n)^2 / bw), B likewise for y (c=2/sqrt(pi)
        # when USE_DERF; the c^2 in numerator and S cancels, eps is scaled).
        tmp = ctx.enter_context(tc.tile_pool(name="tmp", bufs=4))
        for a in range(NT):
            for h_or_v in range(2):  # 0 -> A (x), 1 -> B (y)
                dst = (a_all if h_or_v == 0 else b_all)[:, a * GRID : (a + 1) * GRID]
                bias_ap = negpos_sb[:, 2 * a + h_or_v : 2 * a + h_or_v + 1]
                if USE_DERF:
                    nc.scalar.activation(
                        dst, gridb_sb[:], AF.Derivative_Erf,
                        bias=bias_ap, scale=1.0 / np.sqrt(BW),
                    )
                else:
                    t = tmp.tile([128, GRID], F32, tag="sq")
                    nc.scalar.activation(t[:], gridb_sb[:], AF.Square, bias=bias_ap)
                    nc.scalar.activation(dst, t[:], AF.Exp, scale=-1.0 / BW)

        # normalizer S = A^T @ B (fp32 accum), R = 1/(S+eps)
        with tc.tile_pool(name="ps_s", bufs=1, space="PSUM") as ps_s:
            ps = ps_s.tile([GRID, GRID], F32, tag="ps_s")
            for a in range(NT):
                nc.tensor.matmul(
                    ps[:],
                    a_all[:, a * GRID : (a + 1) * GRID],
                    b_all[:, a * GRID : (a + 1) * GRID],
                    start=(a == 0),
                    stop=(a == NT - 1),
                )
            eps_eff = EPS * DERF_C2 if USE_DERF else EPS
            nc.vector.tensor_scalar_add(s_sb[:], ps[:], eps_eff)
        nc.vector.reciprocal(r_sb[:], s_sb[:])
        # repartition R [64i, 64j] -> [128 part, 32 g-tiles] via DRAM
        nc.sync.dma_start(scr[:].rearrange("(i j) -> i j", i=GRID), r_sb[:])
        nc.sync.dma_start(r_t[:], scr[:].rearrange("(t p) -> p t", p=128))

        # ---------------- phase B: MLP ----------------
        hT = ctx.enter_context(tc.tile_pool(name="hT", bufs=2))
        with (
            tc.tile_pool(name="ps_h", bufs=2, space="PSUM") as ps_h,
            tc.tile_pool(name="ps_p", bufs=2, space="PSUM") as ps_p,
        ):
            for c in range(NCHUNK):
                n0 = c * 512
                # MM1 + exact GELU: hT[m] = gelu(W1^T statesT + b1), [h=512,n=512]
                hts = [hT.tile([128, 512], BF16, tag=f"hT{m}", name=f"hT{m}") for m in range(4)]
                for m in range(4):
                    ph = ps_h.tile([128, 512], F32, tag="ph")
                    for k in range(2):
                        nc.tensor.matmul(
                            ph[:],
                            w1k[k][:, m * 128 : (m + 1) * 128],
                            stb[k][:, n0 : n0 + 512],
                            start=(k == 0),
                            stop=(k == 1),
                        )
                    nc.scalar.activation(
                        hts[m][:], ph[:], AF.Gelu, bias=b1_sb[:, m : m + 1]
                    )
                # MM2 + bias: proj[n_tile] = hT^T W2 + b2, [n=128, o=256]
                for s in range(CSUB):
                    a = c * CSUB + s
                    pp = ps_p.tile([128, O], F32, tag="pp")
                    for k in range(4):
                        nc.tensor.matmul(
                            pp[:],
                            hts[k][:, s * 128 : (s + 1) * 128],
                            w2k[k][:],
                            start=(k == 0),
                            stop=(k == 3),
                        )
                    nc.vector.tensor_add(
                        proj[:, a * O : (a + 1) * O], pp[:], b2b_sb[:]
                    )

        # ---------------- phase C: pooling out = (w_u^T proj) * R ----------
        # proj tiles are the matmul stationaries (4 LDWEIGHTS per (gc,a)
        # instead of 8, 512-row streams instead of 256): accumulate
        # out2[o, g] in PSUM, then PE-transpose 128x128 tiles back to [g, o]
        # and scale by R on the Scalar engine.
        wup = ctx.enter_context(tc.tile_pool(name="wup", bufs=8))
        o2p = ctx.enter_context(tc.tile_pool(name="o2p", bufs=2))
        osbp = ctx.enter_context(tc.tile_pool(name="osbp", bufs=4))
        with (
            tc.tile_pool(name="ps_acc", bufs=1, space="PSUM") as ps_acc,
            tc.tile_pool(name="ps_tr", bufs=4, space="PSUM") as ps_tr,
        ):
            for gc in range(GCHUNK):
                accs = [ps_acc.tile([128, GG], F32, tag=f"acc{h}", name=f"acc{h}") for h in range(2)]
                for a in range(NT):
                    wu = wup.tile([128, GG], BF16, tag="wu")
                    i0 = a * GRID + gc * IPC
                    a3 = a_all[:, i0 : i0 + IPC][:, :, None].broadcast_to(
                        [128, IPC, GRID]
                    )
                    b3 = b_all[:, a * GRID : (a + 1) * GRID][:, None, :].broadcast_to(
                        [128, IPC, GRID]
                    )
                    wu3 = wu[:].rearrange("p (i j) -> p i j", i=IPC)
                    nc.vector.tensor_mul(wu3, a3, b3)
                    for h in range(2):
                        for s in range(2):
                            nc.tensor.matmul(
                                accs[h][:, s * 512 : (s + 1) * 512],
                                proj[:, a * O + h * 128 : a * O + (h + 1) * 128],
                                wu[:, s * 512 : (s + 1) * 512],
                                start=(a == 0),
                                stop=(a == NT - 1),
                            )
                o2 = [o2p.tile([128, GG], BF16, tag=f"o2{h}", name=f"o2{h}") for h in range(2)]
                for h in range(2):
                    for q in range(2):
                        nc.scalar.activation(
                            o2[h][:, q * 512 : (q + 1) * 512],
                            accs[h][:, q * 512 : (q + 1) * 512],
                            AF.Copy,
                        )
                for tt in range(8):
                    gt = gc * 8 + tt
                    osb = osbp.tile([128, O], F32, tag="osb")
                    for h in range(2):
                        ptr = ps_tr.tile([128, 128], BF16, tag="ptr")
                        nc.tensor.transpose(
                            ptr[:], o2[h][:, tt * 128 : (tt + 1) * 128], identb[:]
                        )
                        nc.scalar.activation(
                            osb[:, h * 128 : (h + 1) * 128],
                            ptr[:],
                            AF.Copy,
                            scale=r_t[:, gt : gt + 1],
                        )
                    nc.sync.dma_start(out_ap[gt * 128 : (gt + 1) * 128, :], osb[:])


def build_module():
    nc = bacc.Bacc("TRN2", target_bir_lowering=False, debug=False, num_devices=B)
    aps = {
        "statesT": nc.dram_tensor("statesT", (D, N), BF16, kind="ExternalInput").ap(),
        "W1": nc.dram_tensor("W1", (D, H), BF16, kind="ExternalInput").ap(),
        "b1": nc.dram_tensor("b1", (H,), F32, kind="ExternalInput").ap(),
        "W2": nc.dram_tensor("W2", (H, O), BF16, kind="ExternalInput").ap(),
        "b2b": nc.dram_tensor("b2b", (128, O), F32, kind="ExternalInput").ap(),
        "gridb": nc.dram_tensor("gridb", (128, GRID), F32, kind="ExternalInput").ap(),
        "negpos": nc.dram_tensor(
            "negpos", (128, 2 * NT), F32, kind="ExternalInput"
        ).ap(),
        "ident": nc.dram_tensor("ident", (128, 128), BF16, kind="ExternalInput").ap(),
    }
    out_ap = nc.dram_tensor("out", (G, O), F32, kind="ExternalOutput").ap()
    with tile.TileContext(nc) as tc:
        _body(tc, aps, out_ap)
    nc.compile()
    return nc


_NC = None


def _get_nc():
    global _NC
    if _NC is None:
        _NC = build_module()
    return _NC


def make_in_maps(inputs):
    bf16 = mybir.dt.np(BF16)
    states = np.asarray(inputs["entity_states"], np.float32)
    pos = np.asarray(inputs["entity_positions"], np.float32)
    W1 = np.ascontiguousarray(np.asarray(inputs["W1"], np.float32).astype(bf16))
    b1 = np.ascontiguousarray(np.asarray(inputs["b1"], np.float32))
    W2 = np.ascontiguousarray(np.asarray(inputs["W2"], np.float32).astype(bf16))
    b2 = np.asarray(inputs["b2"], np.float32)

    statesT = np.ascontiguousarray(states.transpose(0, 2, 1).astype(bf16))  # [B,D,N]
    g = np.linspace(-1.0, 1.0, GRID).astype(np.float32)
    gridb = np.ascontiguousarray(np.tile(g[None, :], (128, 1)))
    b2b = np.ascontiguousarray(np.tile(b2[None, :], (128, 1)))
    # negpos[p, 2a+c] = -pos[a*128+p, c] (scaled for the d_erf argument)
    npz = (-pos).reshape(B, NT, 128, 2).transpose(0, 2, 1, 3).reshape(B, 128, 2 * NT)
    if USE_DERF:
        npz = npz / np.sqrt(BW)
    negpos = np.ascontiguousarray(npz.astype(np.float32))
    ident = np.eye(128, dtype=np.float32).astype(bf16)
    return [
        {
            "statesT": statesT[b],
            "W1": W1,
            "b1": b1,
            "W2": W2,
            "b2b": b2b,
            "gridb": gridb,
            "negpos": negpos[b],
            "ident": ident,
        }
        for b in range(B)
    ]


def run(inputs, trace=False, **kw):
    nc = _get_nc()
    res = bass_utils.run_bass_kernel_spmd(
        nc, make_in_maps(inputs), core_ids=list(range(B)), trace=trace, **kw
    )
    out = np.stack([r["out"] for r in res.results], axis=0)
    return out, res


def kernel(**inputs) -> np.ndarray:
    out, _ = run(inputs, trace=False)
    return out
